# revision 24
# baseline (speedup 1.0000x reference)
"""AllegroQeqLayer Trainium2 kernel (8 NeuronCores, SPMD edge-sharded).

Structure:
  NEFF1 (device): per-edge MLP heads -> chi/sig/eps per-edge scalars
                  (channel-major [3,512] tiles), bf16 matmuls, f32 psum.
  host:           segment-sum via np.bincount (irregular scatter), node math
                  (charges/pot/vdw/w), gather w[senders] via np.take.
  NEFF2 (device): final 3-layer edge MLP  h3 = W3(silu(W2(silu(W1 [x|w_se]))))
                  channel-major, 4-tile-stacked PSUM so ACT/DVE run full-width.
  host:           x_out = envelope(|vectors|) * h3^T, assemble outputs.

Everything irregular (scatter/gather over arbitrary senders) is host-side;
everything O(E*D) regular compute is device-side.
"""

import os
import sys

sys.path.insert(0, "/opt/trn_rl_repo")

import numpy as np

import concourse.bass as bass
from concourse import bacc
import concourse.mybir as mybir
import concourse.tile as tile
from concourse.bass_utils import run_bass_kernel_spmd

# ---- problem constants (hardcoded per task statement) ----
N_NODES = 50000
N_EDGES = 1600000
D = 64
CE = 16
HID = 32
NCORES = 8
EC = N_EDGES // NCORES          # 200000 edges per core
F = 512                          # edge tile (moving dim)
EPAD = 204800                    # EC padded to multiple of F
NT = EPAD // F                   # 400 tiles per core
GRP = 4                          # NEFF2: tiles stacked per PSUM group
NG = NT // GRP                   # 100 groups

F32 = mybir.dt.float32

# matmul dtype mode: "bf16" (fast, ~3e-3 rel err) or "f32" (exact, 4x slower PE)
MM_MODE = os.environ.get("KERNEL_MM_MODE", "bf16")
ST = mybir.dt.bfloat16 if MM_MODE == "bf16" else F32

import ml_dtypes

NPST = ml_dtypes.bfloat16 if MM_MODE == "bf16" else np.float32


def _mm_view(ap):
    return ap


# ------------------------------------------------------------------
# NEFF builders
# ------------------------------------------------------------------

def build_neff1():
    nc = bacc.Bacc()
    xt = nc.declare_dram_parameter("xt", [NT, D, F], ST, isOutput=False)
    wcat = nc.declare_dram_parameter("wcat", [D, 80], ST, isOutput=False)
    w2cat = nc.declare_dram_parameter("w2cat", [80, HID], ST, isOutput=False)
    gsel = nc.declare_dram_parameter("gsel", [128, 12], ST, isOutput=False)
    vals = nc.declare_dram_parameter("vals", [NT, 3, F], F32, isOutput=True)

    with tile.TileContext(nc) as tc:
        with (
            tc.tile_pool(name="const", bufs=1) as cpool,
            tc.tile_pool(name="sbuf", bufs=3) as pool,
            tc.tile_pool(name="act", bufs=4) as apool,
            tc.tile_pool(name="out", bufs=3) as opool,
            tc.tile_pool(name="psum", bufs=4, space="PSUM") as pp1,
            tc.tile_pool(name="psum2", bufs=2, space="PSUM") as pp2,
            tc.tile_pool(name="psum3", bufs=2, space="PSUM") as pp3,
        ):
            wc = cpool.tile([D, 80], ST)
            _dma(nc).dma_start(out=wc[:], in_=wcat[:, :])
            w2 = cpool.tile([80, HID], ST)
            _dma(nc).dma_start(out=w2[:], in_=w2cat[:, :])
            gs = cpool.tile([128, 12], ST)
            _dma(nc).dma_start(out=gs[:], in_=gsel[:, :])

            xs_t, p1_t, s_t, v_t = {}, {}, {}, {}

            def stage_a(t):
                g, i = divmod(t, GRP)
                if i == 0:
                    xs = pool.tile([D, GRP, F], ST, tag="xs")
                    _dma(nc).dma_start(
                        out=xs[:],
                        in_=xt[GRP * g : GRP * (g + 1)].rearrange(
                            "t c f -> c t f"
                        ),
                    )
                    xs_t[g] = xs
                p1 = pp1.tile([80, F], F32, tag="p1")
                nc.tensor.matmul(
                    out=p1[:], lhsT=_mm_view(wc[:]),
                    rhs=_mm_view(xs_t[t // GRP][:, i, :]),
                    start=True, stop=True,
                )
                p1_t[t] = p1

            def stage_b(t):
                s = apool.tile([80, F], ST, tag="s")
                nc.scalar.activation(
                    s[:], p1_t.pop(t)[:], mybir.ActivationFunctionType.Silu
                )
                s_t[t] = s

            def stage_c(t):
                g, i = divmod(t, GRP)
                if i == 0:
                    v_t[g] = pp2.tile([128, F], F32, tag="p2", name="p2")
                nc.tensor.matmul(
                    out=v_t[g][32 * i : 32 * (i + 1), :],
                    lhsT=_mm_view(w2[:]), rhs=_mm_view(s_t.pop(t)[:]),
                    start=True, stop=True, tile_position=(0, 32 * i),
                )
                if i == GRP - 1:
                    vb = opool.tile([128, F], ST, tag="vb", name="vb")
                    nc.vector.tensor_copy(vb[:], v_t.pop(g)[:])
                    pc = pp3.tile([12, F], F32, tag="pc", name="pc")
                    nc.tensor.matmul(
                        out=pc[:], lhsT=_mm_view(gs[:]), rhs=_mm_view(vb[:]),
                        start=True, stop=True,
                    )
                    vc = opool.tile([12, F], F32, tag="vc", name="vc")
                    nc.vector.tensor_copy(vc[:], pc[:])
                    _dma(nc).dma_start(
                        out=vals[GRP * g : GRP * (g + 1)].rearrange(
                            "t c f -> (t c) f"
                        ),
                        in_=vc[:],
                    )

            for t in range(NT + 2):
                if t < NT:
                    stage_a(t)
                if 1 <= t <= NT:
                    stage_b(t - 1)
                if t >= 2:
                    stage_c(t - 2)
    return nc


def build_neff2():
    nc = bacc.Bacc()
    xw = nc.declare_dram_parameter("xw", [NT, D + CE, F], ST, isOutput=False)
    wx1 = nc.declare_dram_parameter("wx1", [D + CE, HID], ST, isOutput=False)
    wx2 = nc.declare_dram_parameter("wx2", [GRP * HID, GRP * HID], ST, isOutput=False)
    wx3 = nc.declare_dram_parameter("wx3", [GRP * HID, GRP * HID], ST, isOutput=False)
    h3o = nc.declare_dram_parameter("h3o", [NG, 128, F], F32, isOutput=True)

    with tile.TileContext(nc) as tc:
        with (
            tc.tile_pool(name="const", bufs=1) as cpool,
            tc.tile_pool(name="rhs", bufs=4) as rpool,
            tc.tile_pool(name="h", bufs=6) as hpool,
            tc.tile_pool(name="out", bufs=3) as opool,
            tc.tile_pool(name="ps1", bufs=2, space="PSUM") as pp1,
            tc.tile_pool(name="ps2", bufs=2, space="PSUM") as pp2,
            tc.tile_pool(name="ps3", bufs=2, space="PSUM") as pp3,
        ):
            w1 = cpool.tile([D + CE, HID], ST)
            _dma(nc).dma_start(out=w1[:], in_=wx1[:, :])
            w2 = cpool.tile([GRP * HID, GRP * HID], ST)
            _dma(nc).dma_start(out=w2[:], in_=wx2[:, :])
            w3 = cpool.tile([GRP * HID, GRP * HID], ST)
            _dma(nc).dma_start(out=w3[:], in_=wx3[:, :])

            rt_t, p1_t, h1_t, p2_t, h2_t = {}, {}, {}, {}, {}

            def stage_a(g):
                rt = rpool.tile([D + CE, GRP, F], ST, tag="rt")
                _dma(nc).dma_start(
                    out=rt[:],
                    in_=xw[GRP * g : GRP * (g + 1)].rearrange("t c f -> c t f"),
                )
                p1 = pp1.tile([128, F], F32, tag="p1")
                for i in range(GRP):
                    nc.tensor.matmul(
                        out=p1[32 * i : 32 * (i + 1), :],
                        lhsT=_mm_view(w1[:]), rhs=_mm_view(rt[:, i, :]),
                        start=True, stop=True, tile_position=(0, 32 * i),
                    )
                rt_t[g] = rt
                p1_t[g] = p1

            def stage_b(g):
                rt_t.pop(g)
                h1 = hpool.tile([128, F], ST, tag="h1")
                nc.scalar.activation(
                    h1[:], p1_t.pop(g)[:], mybir.ActivationFunctionType.Silu
                )
                p2 = pp2.tile([128, F], F32, tag="p2")
                nc.tensor.matmul(
                    out=p2[:], lhsT=_mm_view(w2[:]), rhs=_mm_view(h1[:]),
                    start=True, stop=True,
                )
                h1_t[g] = h1
                p2_t[g] = p2

            def stage_c(g):
                h1_t.pop(g)
                h2 = hpool.tile([128, F], ST, tag="h2")
                nc.scalar.activation(
                    h2[:], p2_t.pop(g)[:], mybir.ActivationFunctionType.Silu
                )
                p3 = pp3.tile([128, F], F32, tag="p3")
                nc.tensor.matmul(
                    out=p3[:], lhsT=_mm_view(w3[:]), rhs=_mm_view(h2[:]),
                    start=True, stop=True,
                )
                h2_t[g] = h2
                h3 = opool.tile([128, F], F32, tag="h3")
                nc.vector.tensor_copy(h3[:], p3[:])
                _dma(nc).dma_start(out=h3o[g], in_=h3[:])
                h2_t.pop(g)

            for g in range(NG + 2):
                if g < NG:
                    stage_a(g)
                if 1 <= g <= NG:
                    stage_b(g - 1)
                if g >= 2:
                    stage_c(g - 2)
    return nc


_CACHE = {}


def _get_nc(which):
    if which not in _CACHE:
        nc = build_neff1() if which == 1 else build_neff2()
        if not nc.is_finalized():
            nc.finalize()
        _CACHE[which] = nc
    return _CACHE[which]


LAST_EXEC_NS = {}


def _run(nc, in_maps, label):
    trace = os.environ.get("KERNEL_TRACE", "0") == "1"
    try:
        res = run_bass_kernel_spmd(nc, in_maps, list(range(NCORES)), trace=trace)
    except ModuleNotFoundError:
        res = run_bass_kernel_spmd(nc, in_maps, list(range(NCORES)), trace=False)
    if getattr(res, "exec_time_ns", None) is not None:
        LAST_EXEC_NS[label] = res.exec_time_ns
    return res.results


# ------------------------------------------------------------------
# host helpers
# ------------------------------------------------------------------

def _sigmoid(v):
    out = np.empty_like(v)
    pos = v >= 0
    out[pos] = 1.0 / (1.0 + np.exp(-v[pos]))
    ev = np.exp(v[~pos])
    out[~pos] = ev / (1.0 + ev)
    return out


def _softplus(v):
    return np.maximum(v, 0.0) + np.log1p(np.exp(-np.abs(v)))


def _blockdiag(w):
    b = np.zeros((GRP * HID, GRP * HID), np.float32)
    for i in range(GRP):
        b[HID * i : HID * (i + 1), HID * i : HID * (i + 1)] = w
    return b.astype(NPST)


def _shard_pad_T(arr, c, width):
    """core c's edge shard of [E, width] -> [NT, width, F] (transposed tiles)."""
    sh = arr[c * EC : (c + 1) * EC]
    buf = np.zeros((EPAD, width), np.float32)
    buf[:EC] = sh
    return np.ascontiguousarray(
        buf.reshape(NT, F, width).transpose(0, 2, 1)
    ).astype(NPST)


def kernel(vectors, x, V, senders, species, radius, hardness, charge_embed,
           W_chi1, W_chi2, W_sig1, W_sig2, W_eps1, W_eps2, W_w1,
           W_x1, W_x2, W_x3):
    vectors = np.asarray(vectors, np.float32)
    x = np.asarray(x, np.float32)
    V = np.asarray(V)
    senders = np.asarray(senders).astype(np.int64).ravel()
    species = np.asarray(species).astype(np.int64).ravel()
    radius = np.asarray(radius, np.float32)
    hardness = np.asarray(hardness, np.float32)
    charge_embed = np.asarray(charge_embed, np.float32)

    # ---- fold 1/sqrt(fan_in) scales into weights, build fused mats ----
    s64 = 1.0 / np.sqrt(64.0)
    wcat = np.concatenate(
        [W_chi1 * s64, W_sig1 * s64, W_eps1 * s64], axis=1
    ).astype(NPST)                                          # [64, 80]
    w2cat = np.zeros((80, HID), np.float32)  # cols 3..31 zero
    w2cat[0:16, 0] = np.asarray(W_chi2)[:, 0] / np.sqrt(16.0)
    w2cat[16:48, 1] = np.asarray(W_sig2)[:, 0] / np.sqrt(32.0)
    w2cat[48:80, 2] = np.asarray(W_eps2)[:, 0] / np.sqrt(32.0)
    wx1 = (np.asarray(W_x1) / np.sqrt(80.0)).astype(NPST)   # [80, 32]
    wx2 = (np.asarray(W_x2) / np.sqrt(32.0)).astype(NPST)
    wx3 = (np.asarray(W_x3) / np.sqrt(32.0)).astype(NPST)

    # ---- NEFF1: per-edge chi/sig/eps ----
    gsel = np.zeros((128, 12), np.float32)
    for i in range(GRP):
        for c in range(3):
            gsel[32 * i + c, 3 * i + c] = 1.0
    gsel = gsel.astype(NPST)
    xt_shards = [_shard_pad_T(x, c, D) for c in range(NCORES)]
    in1 = [
        {"xt": xt_shards[c], "wcat": wcat, "w2cat": w2cat.astype(NPST),
         "gsel": gsel}
        for c in range(NCORES)
    ]
    res1 = _run(_get_nc(1), in1, "neff1")

    # ---- host: segment sums + node math ----
    chis = np.zeros(N_NODES, np.float64)
    sigs = np.zeros(N_NODES, np.float64)
    epss = np.zeros(N_NODES, np.float64)
    for c in range(NCORES):
        v = np.asarray(res1[c]["vals"], np.float64)         # [NT, 3, F]
        v = v.transpose(1, 0, 2).reshape(3, EPAD)[:, :EC]
        snd = senders[c * EC : (c + 1) * EC]
        chis += np.bincount(snd, weights=v[0], minlength=N_NODES)
        sigs += np.bincount(snd, weights=v[1], minlength=N_NODES)
        epss += np.bincount(snd, weights=v[2], minlength=N_NODES)
    chis = chis.astype(np.float32)
    sigs = sigs.astype(np.float32)
    epss = epss.astype(np.float32)

    gammas = 4.0 * radius[species] + 0.5
    hard = _softplus(hardness[species].astype(np.float64)).astype(np.float32)
    charges = (-chis / hard).astype(np.float32)
    pot = np.float32(
        0.5 * np.sum((hard + 1.0 / gammas) * charges.astype(np.float64) ** 2)
        + np.sum(chis.astype(np.float64) * charges)
    )
    sigma = _sigmoid(sigs) * np.float32(0.15) + np.float32(0.15)
    eps_n = _sigmoid(epss) * np.float32(1.7) + np.float32(0.3)
    vdw = np.float32(np.sum(eps_n.astype(np.float64) * sigma))

    w_in = np.concatenate([charges[:, None], charge_embed[species]], axis=1)
    w_node = (w_in @ (W_w1 / np.sqrt(1.0 + CE))).astype(np.float32)  # [N,16]
    wg = w_node[senders]                                             # [E,16]

    # ---- NEFF2: final edge MLP ----
    in2 = [
        {
            "xw": np.concatenate(
                [xt_shards[c], _shard_pad_T(wg, c, CE)], axis=1
            ),
            "wx1": wx1, "wx2": _blockdiag(wx2), "wx3": _blockdiag(wx3),
        }
        for c in range(NCORES)
    ]
    res2 = _run(_get_nc(2), in2, "neff2")

    # ---- host: envelope * h3, assemble ----
    lengths = np.sqrt(np.sum(vectors.astype(np.float64) ** 2, axis=-1))
    u = lengths.astype(np.float32)
    env = np.where(
        u < 1.0,
        1.0 + (-28.0) * u**6 + 48.0 * u**7 + (-21.0) * u**8,
        0.0,
    ).astype(np.float32)

    x_out = np.empty((N_EDGES, HID), np.float32)
    for c in range(NCORES):
        h = np.asarray(res2[c]["h3o"], np.float32)          # [NG, 128, F]
        h = h.reshape(NG, GRP, HID, F).transpose(0, 1, 3, 2)  # [NG,GRP,F,32]
        x_out[c * EC : (c + 1) * EC] = h.reshape(EPAD, HID)[:EC]
    x_out *= env[:, None]

    return (x_out, V, charges, pot, vdw)
